# revision 33
# baseline (speedup 1.0000x reference)
"""MultiHeadedAttention Trainium2 Bass kernel (v2).

Full inputs in, full output out. Sharding: 8 cores = 4 batches x 2 head-pairs
(data-parallel over batch, tensor-parallel over the 4 heads).

Per core (batch b, heads h0/h1), all matmuls bf16 except the fp8 x-accum:
  - Q/K proj -> q_sb/k_sb [128 (h,d), 2048] bf16; bias folded into the
    mandatory PSUM->SBUF copy (ACT Identity with per-partition bias AP /
    DVE tensor_scalar_add).
  - V proj -> vt [128 m, 8 j, 2 h, 2 i, 80] fp8e4 (DoubleRow pair layout,
    i = m-block//8, pair (j, j+8); col 64 = ones row for softmax sums).
  - scores: row-tiled matmul pairs via tile_position (h0 rows 0:63,
    h1 rows 64:127) -> two adjacent PSUM banks [128, 1024].
  - exp: ONE instruction per (c4, mb) covering both heads' banks.
    ACT: exp(0.125*s + ln2) -> fp8e4.  DVE: Schraudolph bit-trick
    int8(s/ln2 + 63.8) bitcast as fp8e4 (same 2x scale; scale cancels in
    softmax). Alternating per pair-slot j.
  - x-accum: fp8 DoubleRow matmul per (h, j): K=256 (m-blocks j and j+8),
    M=65 (64 d + ones), N=512. PSUM accumulates over j; row 64 = sums.
  - normalize: copy px->SBUF, broadcast sums row via SBUF->SBUF DMA,
    reciprocal_approx_fast, multiply -> xcat [128 (h,d), 512] bf16.
  - out-proj: lhsT=wmcat [128 (h,d), 128 oc] (heads fused, K=128),
    streamed per 512-wide n chunk; out fp32, host adds pair partials + bias.
"""

import sys

if "/opt/trn_rl_repo" not in sys.path:
    sys.path.insert(0, "/opt/trn_rl_repo")

import numpy as np
import ml_dtypes

BF = ml_dtypes.bfloat16
F8 = ml_dtypes.float8_e4m3fn

B, D, N, H = 4, 256, 2048, 4
DIM = D // H  # 64
NW = 4   # 512-wide input windows
MB = 16  # 128-wide m blocks
LN2 = 0.6931471805599453
SCHRAUD_A = 1.0 / LN2          # bits = A*s + B  (score -> fp8e4 bit pattern)
SCHRAUD_B = 63.8               # 64 = x2 scale (matches ACT's +ln2 bias)

_CACHE = {}

import os
CFG_DR = os.environ.get("K_DR", "1") == "1"          # DoubleRow x-accum
CFG_SCHRAUD = os.environ.get("K_SCHRAUD", "1") == "1"  # DVE bit-trick exp
CFG_ACT_FP8 = os.environ.get("K_ACT_FP8", "1") == "1"  # ACT exp -> fp8 out
CFG_TILEPOS = os.environ.get("K_TILEPOS", "1") == "1"  # row-tiled scores
CFG_FILLER = os.environ.get("K_FILLER", "0") == "1"   # HAM keep-warm fillers
CFG_WARMUP = int(os.environ.get("K_WARMUP", "8"))


def _emit(ctx, tc, io):
    import concourse.bass as bass
    import concourse.mybir as mybir

    nc = tc.nc
    f32 = mybir.dt.float32
    bf16 = mybir.dt.bfloat16
    fp8 = mybir.dt.float8e4
    i8 = mybir.dt.int8
    EXP = mybir.ActivationFunctionType.Exp
    IDENT = mybir.ActivationFunctionType.Identity
    MUL = mybir.AluOpType.mult
    ADD = mybir.AluOpType.add
    DR = mybir.MatmulPerfMode.DoubleRow

    const = ctx.enter_context(tc.tile_pool(name="const", bufs=1))
    xin = ctx.enter_context(tc.tile_pool(name="xin", bufs=12))
    big = ctx.enter_context(tc.tile_pool(name="big", bufs=1))
    ptp = ctx.enter_context(tc.tile_pool(name="ptp", bufs=2))
    work = ctx.enter_context(tc.tile_pool(name="work", bufs=4))
    xcp = ctx.enter_context(tc.tile_pool(name="xcp", bufs=2))
    outp = ctx.enter_context(tc.tile_pool(name="outp", bufs=4))
    psSC = ctx.enter_context(tc.tile_pool(name="psSC", bufs=2, space="PSUM"))
    psPX = ctx.enter_context(tc.tile_pool(name="psPX", bufs=2, space="PSUM"))
    psPR = ctx.enter_context(tc.tile_pool(name="psPR", bufs=2, space="PSUM"))

    # ---- weights (scalar + gpsimd queues; small, first) ----
    wqt_sb = const.tile([128, 2, 128], bf16, tag="wqt")
    nc.scalar.dma_start(wqt_sb, io["wqt"])
    wkt_sb = const.tile([128, 2, 128], bf16, tag="wkt")
    nc.scalar.dma_start(wkt_sb, io["wkt"])
    bq_sb = const.tile([128, 1], f32, tag="bq")
    nc.scalar.dma_start(bq_sb, io["bq"])
    bk_sb = const.tile([128, 1], f32, tag="bk")
    nc.scalar.dma_start(bk_sb, io["bk"])
    wvt_sb = const.tile([128, 2, 128], bf16, tag="wvt")
    nc.gpsimd.dma_start(wvt_sb, io["wvt"])
    bv_sb = const.tile([1, 128], bf16, tag="bv")
    nc.gpsimd.dma_start(bv_sb, io["bv"])
    wmcat_sb = const.tile([128, 2, 256], bf16, tag="wmcat")
    nc.gpsimd.dma_start(wmcat_sb, io["wmcat"])
    onesb = const.tile([1, 128], bf16, tag="onesb")
    nc.gpsimd.memset(onesb, 1.0)
    ones_f = const.tile([128, 64], f32, tag="ones_f")
    nc.gpsimd.memset(ones_f, 1.0)
    ones_r = const.tile([128, 64], mybir.dt.float32r, tag="ones_r")
    nc.vector.tensor_copy(ones_r, ones_f)
    ln2b = const.tile([128, 1], f32, tag="ln2b")
    nc.gpsimd.memset(ln2b, LN2)

    # ---- PE warmup: garbage matmuls release the HAM clock gate while the
    # input DMAs stream. Values are never read. ----
    wu = const.tile([128, 512], bf16, tag="wu")
    nc.vector.memset(wu[0:1, 0:1], 0.0)  # allocate; rest is garbage, never read
    wu_ps = psSC.tile([128, 1024], f32, tag="sc", name="wu_ps")
    for _ in range(CFG_WARMUP):
        nc.tensor.matmul(wu_ps[:, 0:512], lhsT=wu[:, 0:128], rhs=wu,
                         start=True, stop=True)

    # ---- input DMAs: window-contiguous host layout, 2KB/partition per
    # window. K first (scores m loop), Q w0 (first n chunk), V next. ----
    xq_t, xk_t, xv_t = [None] * NW, [None] * NW, [None] * NW
    order = [("xk", 0), ("xq", 0), ("xv", 0), ("xk", 1), ("xv", 1),
             ("xk", 2), ("xv", 2), ("xk", 3), ("xv", 3),
             ("xq", 1), ("xq", 2), ("xq", 3)]
    tiles = {"xq": xq_t, "xk": xk_t, "xv": xv_t}
    engs = [nc.sync, nc.gpsimd]
    for n_i, (name, w) in enumerate(order):
        t = xin.tile([128, 2, 512], bf16, tag=name, name=f"{name}{w}")
        engs[n_i % 2].dma_start(t, io[name][:, w, :, :])
        tiles[name][w] = t

    # ---- vt tile: cols 0:64 = v, col 64 = ones (softmax sums -> px row
    # 64), cols 65:79 zero pad (16-aligned DoubleRow weights AP, M=80;
    # px rows 65:79 dead). DR pair slot j covers m-blocks (2j, 2j+1),
    # i = mb % 2. ----
    vt = big.tile([128, 8, 2, 2, 80], fp8, tag="vt")
    nc.gpsimd.memset(vt, 0.0)
    for h in range(2):
        for i in range(2):
            nc.gpsimd.memset(vt[:, :, h, i, 64:65], 1.0)

    q_sb = big.tile([128, 2048], bf16, tag="q")
    k_sb = big.tile([128, 2048], bf16, tag="k")

    # ---- projections (PE emission order = DMA arrival order) ----
    def qk_proj(xt, wt, bias, dst, w, use_act):
        ps = psPR.tile([128, 512], f32, tag="pr", name=f"ps_{dst.name}{w}")
        nc.tensor.matmul(ps, lhsT=wt[:, 0, :], rhs=xt[w][:, 0, :],
                         start=True, stop=False)
        nc.tensor.matmul(ps, lhsT=wt[:, 1, :], rhs=xt[w][:, 1, :],
                         start=False, stop=True)
        ws = slice(w * 512, (w + 1) * 512)
        if use_act:
            nc.scalar.activation(dst[:, ws], ps, IDENT, bias=bias, scale=1.0)
        else:
            nc.vector.tensor_scalar_add(dst[:, ws], ps, bias)

    def v_proj(w, use_act):
        # 4 m-blocks (mb = 4w+r, pair slots j = 2w, 2w+1) -> one [128, 512]
        # psum -> two fp8 copies (one per i parity) into vt.
        ps = psPR.tile([128, 512], f32, tag="pr", name=f"ps_v{w}")
        for r in range(4):
            ms = slice(r * 128, (r + 1) * 128)
            pvt = ps[:, r * 128:(r + 1) * 128]
            nc.tensor.matmul(pvt, lhsT=onesb, rhs=bv_sb, start=True, stop=False)
            nc.tensor.matmul(pvt, lhsT=xv_t[w][:, 0, ms], rhs=wvt_sb[:, 0, :],
                             start=False, stop=False)
            nc.tensor.matmul(pvt, lhsT=xv_t[w][:, 1, ms], rhs=wvt_sb[:, 1, :],
                             start=False, stop=True)
        src = ps[:, :].rearrange("m (r h d) -> m r h d", r=4, h=2)
        for i in range(2):
            dst = vt[:, 2 * w:2 * w + 2, :, i, 0:64]
            if use_act:
                nc.scalar.copy(dst, src[:, i::2, :, :])
            else:
                nc.vector.tensor_copy(dst, src[:, i::2, :, :])

    # ---- attention: c4-outer (4 x 512-wide n chunks), mb inner.
    # K/V projections for windows 1..3 are emitted inside the c4=0 loop
    # (scores for m-window w only need them by mb=4w); Q windows 1..3 are
    # emitted at the ends of chunks 0..2. Tail work of chunk c4 is emitted
    # interleaved into chunk c4+1's mb loop to avoid head-of-line blocking
    # on the in-order engines. ----
    qk_proj(xk_t, wkt_sb, bk_sb, k_sb, 0, True)
    qk_proj(xq_t, wqt_sb, bq_sb, q_sb, 0, False)
    v_proj(0, True)

    deferred = {}  # (c4, mb) -> list of callables

    def run_deferred(pos):
        for fn in deferred.pop(pos, []):
            fn()

    def make_tail(c4, px, xcat):
        cs = slice(c4 * 512, (c4 + 1) * 512)
        state = {}

        def cp_step(h, use_act):
            def fn():
                cp = work.tile([65, 512], mybir.dt.float32r, tag="cp",
                               name=f"cp{c4}_{h}")
                if use_act:
                    nc.scalar.copy(cp, px[h][0:65, :])
                else:
                    nc.vector.tensor_copy(cp, px[h][0:65, :])
                state[h] = cp
            return fn

        def bcast_step(h):
            def fn():
                # PE broadcast of the sums row (cp row 64, partition 64;
                # explicit tile_position since lhsT base is 0)
                rb = psPR.tile([128, 512], f32, tag="pr", name=f"rb{c4}_{h}")
                nc.tensor.matmul(
                    rb[0:64, :], lhsT=ones_r[64:65, :], rhs=state[h][64:65, :],
                    start=True, stop=True, tile_position=(64, 0))
                state[(h, "rb")] = rb
            return fn

        def norm_step(h):
            def fn():
                cp = state[h]
                rb = state[(h, "rb")]
                rc = work.tile([64, 512], f32, tag="rc", name=f"rc{c4}_{h}")
                nc.vector.reciprocal_approx_fast(rc, rb[0:64, :])
                nc.vector.tensor_tensor(
                    xcat[h][0:64, :], cp[0:64, :].bitcast(f32),
                    rc, op=MUL)
            return fn

        def oproj_step(oc, use_act):
            def fn():
                po = psPR.tile([128, 512], f32, tag="pr", name=f"po{c4}_{oc}")
                for h in range(2):
                    nc.tensor.matmul(
                        po, lhsT=wmcat_sb[0:64, h, oc * 128:(oc + 1) * 128],
                        rhs=xcat[h][0:64, :],
                        start=(h == 0), stop=(h == 1))
                ob = outp.tile([128, 512], f32, tag="ob", name=f"ob{c4}_{oc}")
                if use_act:
                    nc.scalar.copy(ob, po)
                else:
                    nc.vector.tensor_copy(ob, po)
                nc.sync.dma_start(io["out"][oc * 128:(oc + 1) * 128, cs], ob)
            return fn

        return [cp_step(0, True), bcast_step(0), cp_step(1, False),
                bcast_step(1), norm_step(0), norm_step(1),
                oproj_step(0, True), oproj_step(1, False)]

    for w in range(1, NW):
        deferred.setdefault((0, 4 * w), []).extend([
            (lambda w=w: qk_proj(xk_t, wkt_sb, bk_sb, k_sb, w,
                                 use_act=(w % 2 == 0))),
            (lambda w=w: v_proj(w, use_act=(w % 2 == 1))),
        ])

    for c4 in range(4):
        cs = slice(c4 * 512, (c4 + 1) * 512)
        px = [psPX.tile([128, 512], f32, tag="px", name=f"px{c4}_{h}")
              for h in range(2)]
        ptb = ptp.tile([128, 16, 2, 512], fp8, tag="pt", name=f"pt{c4}")
        xcat = [xcp.tile([64, 512], bf16, tag="xc", name=f"xc{c4}_{h}")
                for h in range(2)]

        for mb in range(MB):
            run_deferred((c4, mb))
            sc = psSC.tile([128, 1024], f32, tag="sc", name=f"sc{c4}_{mb}")
            for h in range(2):
                nc.tensor.matmul(
                    sc[:, h * 512:(h + 1) * 512],
                    lhsT=k_sb[h * 64:(h + 1) * 64, mb * 128:(mb + 1) * 128],
                    rhs=q_sb[h * 64:(h + 1) * 64, cs],
                    start=True, stop=True,
                    tile_position=(h * 64, 0) if CFG_TILEPOS else None,
                )
            j = mb // 2
            use_act = ((j + c4) % 2 == 0)
            if not CFG_SCHRAUD:
                use_act = True
            elif not CFG_ACT_FP8:
                use_act = False
            pslice = ptb[:, mb, :, :]
            if use_act:
                nc.scalar.activation(pslice, sc, EXP, scale=0.125, bias=ln2b)
            else:
                nc.vector.tensor_scalar(
                    pslice.bitcast(i8), sc, SCHRAUD_A, SCHRAUD_B, MUL, ADD)
            if mb % 2 == 1:
                for h in range(2):
                    if CFG_DR:
                        nc.tensor.matmul(
                            px[h][0:80, :],
                            lhsT=vt[:, j, h, :, :],
                            rhs=ptb[:, 2 * j:2 * j + 2, h, :],
                            start=(j == 0), stop=(j == 7),
                            perf_mode=DR,
                        )
                    else:
                        for i in range(2):
                            nc.tensor.matmul(
                                px[h][0:80, :],
                                lhsT=vt[:, j, h, i, :],
                                rhs=ptb[:, 2 * j + i, h, :],
                                start=(j == 0 and i == 0),
                                stop=(j == 7 and i == 1),
                            )
            elif CFG_FILLER:
                # writes the dead rows 96:128 of px[0]
                nc.tensor.matmul(
                    px[0][96:128, :],
                    lhsT=k_sb[0:64, mb * 128:mb * 128 + 32],
                    rhs=q_sb[0:64, cs],
                    start=True, stop=True, skip_group_check=True,
                    tile_position=(0, 96) if CFG_TILEPOS else None,
                )

        tail_ops = make_tail(c4, px, xcat)
        if c4 < 3:
            if c4 == 0:
                # window-3 projections land at (1, 0); shift tail later
                positions = [1, 2, 3, 4, 5, 6, 8, 10]
            else:
                positions = [1, 2, 3, 4, 5, 6, 8, 10]
            for k, fn in enumerate(tail_ops):
                deferred.setdefault((c4 + 1, positions[k]), []).append(fn)
        else:
            for fn in tail_ops:
                fn()
        if c4 < 3:
            deferred.setdefault((c4 + 1, 0), []).append(
                lambda w=c4 + 1: qk_proj(xq_t, wqt_sb, bq_sb, q_sb, w,
                                         use_act=(w % 2 == 1)))

    if "dbg_q" in io:
        nc.sync.dma_start(io["dbg_q"], q_sb)
        nc.sync.dma_start(io["dbg_k"], k_sb)
        nc.sync.dma_start(io["dbg_vt"], vt.bitcast(i8))


def _build_nc(debug_dumps=False):
    key = ("nc", debug_dumps)
    if key in _CACHE:
        return _CACHE[key]
    from contextlib import ExitStack

    import concourse.mybir as mybir
    import concourse.tile as tile
    from concourse import bacc

    f32 = mybir.dt.float32
    bf16 = mybir.dt.bfloat16
    i8 = mybir.dt.int8
    nc = bacc.Bacc("TRN2", target_bir_lowering=False, debug=False, num_devices=8)
    io = {}
    for name, shape, dt_ in (
        ("xq", [128, 4, 2, 512], bf16),
        ("xk", [128, 4, 2, 512], bf16),
        ("xv", [128, 4, 2, 512], bf16),
        ("wqt", [128, 2, 128], bf16),
        ("wkt", [128, 2, 128], bf16),
        ("wvt", [128, 2, 128], bf16),
        ("bq", [128, 1], f32),
        ("bk", [128, 1], f32),
        ("bv", [1, 128], bf16),
        ("wmcat", [128, 2, 256], bf16),
    ):
        io[name] = nc.dram_tensor(name, shape, dt_, kind="ExternalInput").ap()
    io["out"] = nc.dram_tensor("out", [256, 2048], f32, kind="ExternalOutput").ap()
    if debug_dumps:
        io["dbg_q"] = nc.dram_tensor("dbg_q", [128, 2048], bf16, kind="ExternalOutput").ap()
        io["dbg_k"] = nc.dram_tensor("dbg_k", [128, 2048], bf16, kind="ExternalOutput").ap()
        io["dbg_vt"] = nc.dram_tensor("dbg_vt", [128, 8, 2, 2, 80], i8, kind="ExternalOutput").ap()

    with tile.TileContext(nc) as tc:
        with ExitStack() as ctx:
            _emit(ctx, tc, io)
    nc.compile()
    _CACHE[key] = nc
    _CACHE[(key, "io")] = io
    return nc


def make_in_maps(query, key, value, wq, bq, wk, bk, wv, bv, wm, bm):
    fb = lambda a: np.ascontiguousarray(np.asarray(a, dtype=np.float32).astype(BF))
    f = lambda a: np.ascontiguousarray(np.asarray(a), dtype=np.float32)
    query, key, value = f(query), f(key), f(value)
    wq, wk, wv, wm = f(wq), f(wk), f(wv), f(wm)
    bq, bk, bv = f(bq), f(bk), f(bv)

    def win(x):
        # [256, 2048] -> [128 p, 4 w, 2 cc, 512] with channel = cc*128 + p
        return fb(x.reshape(2, 128, 4, 512).transpose(1, 2, 0, 3))

    def wt(w, idx):
        # [256 in, 128 out(hd)] -> [128 p, 2 cc, 128 o]
        return fb(w[idx].T.reshape(2, 128, 128).transpose(1, 0, 2))

    def wmcat_host(wm, idx):
        # [128 p, 2 h, 256 o]; rows 0:64 hold head h's wm rows (d = p)
        arr = np.zeros((128, 2, 256), dtype=np.float32)
        wsl = wm[:, idx].T.reshape(2, 64, 256)  # [h, d, o]
        arr[:64, 0, :] = wsl[0]
        arr[:64, 1, :] = wsl[1]
        return fb(arr)

    in_maps = []
    for c in range(8):
        b, pair = divmod(c, 2)
        hs = (2 * pair, 2 * pair + 1)
        idx = np.array([d * H + h for h in hs for d in range(DIM)])
        m = {
            "xq": win(query[b]),
            "xk": win(key[b]),
            "xv": win(value[b]),
            "wqt": wt(wq, idx),
            "wkt": wt(wk, idx),
            "wvt": wt(wv, idx),
            "bq": f(bq[idx].reshape(128, 1)),
            "bk": f(bk[idx].reshape(128, 1)),
            "bv": fb(bv[idx].reshape(1, 128)),
            "wmcat": wmcat_host(wm, idx),
        }
        in_maps.append(m)
    return in_maps


def run(in_maps, trace=False, **kw):
    from concourse import bass_utils

    nc = _build_nc()
    return bass_utils.run_bass_kernel_spmd(
        nc, in_maps, core_ids=list(range(8)), trace=trace, **kw
    )


def gather(results, bm):
    bm = np.asarray(bm, dtype=np.float32)
    outs = [np.asarray(r["out"], dtype=np.float32) for r in results]
    return np.stack([outs[2 * b] + outs[2 * b + 1] + bm[:, None] for b in range(B)])


def kernel(query, key, value, wq, bq, wk, bk, wv, bv, wm, bm):
    in_maps = make_in_maps(query, key, value, wq, bq, wk, bk, wv, bv, wm, bm)
    res = run(in_maps)
    return gather(res.results, bm)
